# revision 1
# baseline (speedup 1.0000x reference)
import sys

sys.path.insert(0, "/opt/trn_rl_repo")

import ml_dtypes
import numpy as np

import concourse.bass as bass  # noqa: F401
import concourse.tile as tile
from concourse import bacc, mybir
from concourse.bass_utils import run_bass_kernel_spmd

# Problem shapes (hardcoded per contract).
N, D = 16384, 2048
NCORES = 8
NSHARD = N // NCORES  # 2048 rows per core
HALF_LOG_2PI = 0.5 * float(np.log(2.0 * np.pi))

P = 128  # SBUF partitions
KC = D // P  # 16 k-chunks of 128 features
NBLK = 512  # n-columns per transposed block (= 1 PSUM bank of f32)
TB = NBLK // P  # 4 row-tiles per block
BLOCKS = NSHARD // NBLK  # 4 blocks per core

_PROGRAM = None
LAST_RESULT = None  # BassKernelResults of the most recent run (for test harness)


def _build_program():
    """out[n] = gamma - sum_k (p[k]*x[n,k] + q[k])**2, computed as:
    PE-transpose x into [k, n] layout, one fused ACT Square(p*x+q) pass
    (per-partition scale/bias = per-feature), then a [-1]*u matmul reduce
    over k accumulated in PSUM.
    """
    nc = bacc.Bacc(
        "TRN2",
        target_bir_lowering=False,
        debug=False,
        enable_asserts=False,
        num_devices=NCORES,
    )
    f32 = mybir.dt.float32
    bf16 = mybir.dt.bfloat16

    x_ap = nc.dram_tensor("x", [NSHARD, D], f32, kind="ExternalInput").ap()
    p_ap = nc.dram_tensor("pcoef", [P, KC], f32, kind="ExternalInput").ap()
    q_ap = nc.dram_tensor("qcoef", [P, KC], f32, kind="ExternalInput").ap()
    no_ap = nc.dram_tensor("negones", [P, 1], bf16, kind="ExternalInput").ap()
    id_ap = nc.dram_tensor("ident", [P, P], f32, kind="ExternalInput").ap()
    g_ap = nc.dram_tensor("gamma", [1, 1], f32, kind="ExternalInput").ap()
    out_ap = nc.dram_tensor("out", [1, NSHARD], f32, kind="ExternalOutput").ap()

    with tile.TileContext(nc) as tc:
        with (
            tc.tile_pool(name="coef", bufs=1) as coef,
            tc.tile_pool(name="xp", bufs=8) as xp,
            tc.tile_pool(name="up", bufs=4) as up,
            tc.tile_pool(name="obuf", bufs=2) as obuf,
            tc.tile_pool(name="pst", bufs=3, space="PSUM") as pst,
            tc.tile_pool(name="psa", bufs=2, space="PSUM") as psa,
        ):
            p_sb = coef.tile([P, KC], f32)
            nc.sync.dma_start(p_sb[:], p_ap[:, :])
            q_sb = coef.tile([P, KC], f32)
            nc.sync.dma_start(q_sb[:], q_ap[:, :])
            no_sb = coef.tile([P, 1], bf16)
            nc.sync.dma_start(no_sb[:], no_ap[:, :])
            id_sb = coef.tile([P, P], f32)
            nc.sync.dma_start(id_sb[:], id_ap[:, :])
            g_sb = coef.tile([1, 1], f32)
            nc.sync.dma_start(g_sb[:], g_ap[:, :])

            for b in range(BLOCKS):
                xts = []
                for t in range(TB):
                    xt = xp.tile([P, D], f32)
                    r0 = (b * TB + t) * P
                    nc.sync.dma_start(xt[:], x_ap[r0 : r0 + P, :])
                    xts.append(xt)
                acc = psa.tile([1, NBLK], f32)
                for c in range(KC):
                    pt = pst.tile([P, NBLK], f32)
                    for t in range(TB):
                        nc.tensor.transpose(
                            pt[:, t * P : (t + 1) * P],
                            xts[t][:, c * P : (c + 1) * P],
                            id_sb[:],
                        )
                    u = up.tile([P, NBLK], bf16)
                    nc.scalar.activation(
                        u[:],
                        pt[:],
                        mybir.ActivationFunctionType.Square,
                        bias=q_sb[:, c : c + 1],
                        scale=p_sb[:, c : c + 1],
                    )
                    nc.tensor.matmul(
                        acc[:],
                        no_sb[:],
                        u[:],
                        start=(c == 0),
                        stop=(c == KC - 1),
                    )
                ob = obuf.tile([1, NBLK], f32)
                nc.vector.tensor_scalar_add(ob[:], acc[:], g_sb[0:1, 0:1])
                nc.sync.dma_start(out_ap[0:1, b * NBLK : (b + 1) * NBLK], ob[:])

    nc.compile()
    return nc


def kernel(x, raw_params, edges, _trace=False):
    global _PROGRAM, LAST_RESULT
    x = np.ascontiguousarray(np.asarray(x, dtype=np.float32))
    raw_params = np.asarray(raw_params, dtype=np.float64)
    edges = np.asarray(edges)
    assert x.shape == (N, D), x.shape

    # Tiny host-side coefficient math (O(D); the O(N*D) pass runs on device).
    means = np.tanh(raw_params[:D]) * 2.0
    stds = np.logaddexp(0.0, raw_params[D:]) + 1e-6  # softplus + eps
    deg = np.zeros(D, dtype=np.float64)
    np.add.at(deg, edges.reshape(-1), 1.0)
    p = np.sqrt(0.5 * deg) / stds
    q = -means * p
    gamma = float(-np.sum(deg * (np.log(stds) + HALF_LOG_2PI)))

    p2d = np.ascontiguousarray(p.reshape(KC, P).T.astype(np.float32))
    q2d = np.ascontiguousarray(q.reshape(KC, P).T.astype(np.float32))
    negones = np.full((P, 1), -1.0, dtype=ml_dtypes.bfloat16)
    ident = np.eye(P, dtype=np.float32)
    g_arr = np.full((1, 1), gamma, dtype=np.float32)

    if _PROGRAM is None:
        _PROGRAM = _build_program()
    nc = _PROGRAM

    in_maps = []
    for c in range(NCORES):
        shard = x[c * NSHARD : (c + 1) * NSHARD]
        in_maps.append(
            {
                "x": shard,
                "pcoef": p2d,
                "qcoef": q2d,
                "negones": negones,
                "ident": ident,
                "gamma": g_arr,
            }
        )

    LAST_RESULT = run_bass_kernel_spmd(
        nc, in_maps, core_ids=list(range(NCORES)), trace=_trace
    )
    out = np.concatenate(
        [LAST_RESULT.results[c]["out"].reshape(-1) for c in range(NCORES)]
    )
    return out.astype(np.float32)


# revision 3
# speedup vs baseline: 10.9856x; 10.9856x over previous
import sys

sys.path.insert(0, "/opt/trn_rl_repo")

import ml_dtypes
import numpy as np

import concourse.bass as bass  # noqa: F401
import concourse.tile as tile
from concourse import bacc, mybir
from concourse.bass_utils import run_bass_kernel_spmd

# Problem shapes (hardcoded per contract).
N, D = 16384, 2048
NCORES = 8
NSHARD = N // NCORES  # 2048 rows per core
HALF_LOG_2PI = 0.5 * float(np.log(2.0 * np.pi))

P = 128  # SBUF partitions
KC = D // P  # 16 k-chunks of 128 features
NBLK = 512  # n-columns per transposed block (= 1 PSUM bank of f32)
TB = NBLK // P  # 4 row-tiles per block
BLOCKS = NSHARD // NBLK  # 4 blocks per core

_PROGRAM = None
LAST_RESULT = None  # BassKernelResults of the most recent run (for test harness)


def _build_program(repeat=1):
    """out[n] = gamma - sum_k (p[k]*x[n,k] + q[k])**2, computed as:
    PE-transpose x into [k, n] layout, one fused ACT Square(p*x+q) pass
    (per-partition scale/bias = per-feature), then a [-1]*u matmul reduce
    over k accumulated in PSUM.

    repeat>1 re-runs the whole pass inside one NEFF (for differential
    HW timing only; results identical).
    """
    nc = bacc.Bacc(
        "TRN2",
        target_bir_lowering=False,
        debug=False,
        enable_asserts=False,
        num_devices=NCORES,
    )
    f32 = mybir.dt.float32
    bf16 = mybir.dt.bfloat16

    x_ap = nc.dram_tensor("x", [NSHARD, D], f32, kind="ExternalInput").ap()
    p_ap = nc.dram_tensor("pcoef", [P, KC], f32, kind="ExternalInput").ap()
    q_ap = nc.dram_tensor("qcoef", [P, KC], f32, kind="ExternalInput").ap()
    no_ap = nc.dram_tensor("negones", [P, 1], bf16, kind="ExternalInput").ap()
    id_ap = nc.dram_tensor("ident", [P, P], f32, kind="ExternalInput").ap()
    g_ap = nc.dram_tensor("gamma", [1, 1], f32, kind="ExternalInput").ap()
    out_ap = nc.dram_tensor("out", [1, NSHARD], f32, kind="ExternalOutput").ap()

    with tile.TileContext(nc) as tc:
        with (
            tc.tile_pool(name="coef", bufs=1) as coef,
            tc.tile_pool(name="xp", bufs=8) as xp,
            tc.tile_pool(name="up", bufs=4) as up,
            tc.tile_pool(name="obuf", bufs=2) as obuf,
            tc.tile_pool(name="pst", bufs=3, space="PSUM") as pst,
            tc.tile_pool(name="psa", bufs=2, space="PSUM") as psa,
        ):
            p_sb = coef.tile([P, KC], f32)
            nc.sync.dma_start(p_sb[:], p_ap[:, :])
            q_sb = coef.tile([P, KC], f32)
            nc.sync.dma_start(q_sb[:], q_ap[:, :])
            no_sb = coef.tile([P, 1], bf16)
            nc.sync.dma_start(no_sb[:], no_ap[:, :])
            id_sb = coef.tile([P, P], f32)
            nc.sync.dma_start(id_sb[:], id_ap[:, :])
            g_sb = coef.tile([1, 1], f32)
            nc.sync.dma_start(g_sb[:], g_ap[:, :])

            for _rep in range(repeat):
              for b in range(BLOCKS):
                xts = []
                for t in range(TB):
                    xt = xp.tile([P, D], f32)
                    r0 = (b * TB + t) * P
                    nc.sync.dma_start(xt[:], x_ap[r0 : r0 + P, :])
                    xts.append(xt)
                acc = psa.tile([1, NBLK], f32)
                for c in range(KC):
                    pt = pst.tile([P, NBLK], f32)
                    for t in range(TB):
                        nc.tensor.transpose(
                            pt[:, t * P : (t + 1) * P],
                            xts[t][:, c * P : (c + 1) * P],
                            id_sb[:],
                        )
                    u = up.tile([P, NBLK], bf16)
                    nc.scalar.activation(
                        u[:],
                        pt[:],
                        mybir.ActivationFunctionType.Square,
                        bias=q_sb[:, c : c + 1],
                        scale=p_sb[:, c : c + 1],
                    )
                    nc.tensor.matmul(
                        acc[:],
                        no_sb[:],
                        u[:],
                        start=(c == 0),
                        stop=(c == KC - 1),
                    )
                ob = obuf.tile([1, NBLK], f32)
                nc.vector.tensor_scalar_add(ob[:], acc[:], g_sb[0:1, 0:1])
                nc.sync.dma_start(out_ap[0:1, b * NBLK : (b + 1) * NBLK], ob[:])

    nc.compile()
    return nc


def kernel(x, raw_params, edges, _trace=False):
    global _PROGRAM, LAST_RESULT
    x = np.ascontiguousarray(np.asarray(x, dtype=np.float32))
    raw_params = np.asarray(raw_params, dtype=np.float64)
    edges = np.asarray(edges)
    assert x.shape == (N, D), x.shape

    # Tiny host-side coefficient math (O(D); the O(N*D) pass runs on device).
    means = np.tanh(raw_params[:D]) * 2.0
    stds = np.logaddexp(0.0, raw_params[D:]) + 1e-6  # softplus + eps
    deg = np.zeros(D, dtype=np.float64)
    np.add.at(deg, edges.reshape(-1), 1.0)
    p = np.sqrt(0.5 * deg) / stds
    q = -means * p
    gamma = float(-np.sum(deg * (np.log(stds) + HALF_LOG_2PI)))

    p2d = np.ascontiguousarray(p.reshape(KC, P).T.astype(np.float32))
    q2d = np.ascontiguousarray(q.reshape(KC, P).T.astype(np.float32))
    negones = np.full((P, 1), -1.0, dtype=ml_dtypes.bfloat16)
    ident = np.eye(P, dtype=np.float32)
    g_arr = np.full((1, 1), gamma, dtype=np.float32)

    if _PROGRAM is None:
        _PROGRAM = _build_program()
    nc = _PROGRAM

    in_maps = []
    for c in range(NCORES):
        shard = x[c * NSHARD : (c + 1) * NSHARD]
        in_maps.append(
            {
                "x": shard,
                "pcoef": p2d,
                "qcoef": q2d,
                "negones": negones,
                "ident": ident,
                "gamma": g_arr,
            }
        )

    LAST_RESULT = run_bass_kernel_spmd(
        nc, in_maps, core_ids=list(range(NCORES)), trace=_trace
    )
    out = np.concatenate(
        [LAST_RESULT.results[c]["out"].reshape(-1) for c in range(NCORES)]
    )
    return out.astype(np.float32)
